# revision 42
# baseline (speedup 1.0000x reference)
"""Trainium2 Bass kernel for nn_Clash_net (clash energy over atom pairs).

Contract: kernel(**inputs) takes FULL (unsharded) numpy inputs as produced by
setup_inputs() and returns the FULL [6] float32 energies output.

Strategy (8 NeuronCores, SPMD over the pair dimension):

The problem is gather-bound: each of the 4M pairs needs two random 16 B
records ([x,y,z,r]) from a 100K-atom table.  The previous kernel used
gpsimd.indirect_dma_start with [128,1] offsets (~24 us per 128 records,
Q7 cross-partition offset reads) -> ~95 ms.  This kernel replaces it with
the vectorized `dma_gather` custom instruction (mlp ucode library): ONE
instruction gathers 896 records by an int16 index list that the Q7 cores
read from their own 16 partitions, emitting descriptors 16-at-a-time with
SIMD pushes.  Measured throughput is ~100x the indirect-DMA path.

dma_gather constraints and how they are met:
  - gathered element size must be a multiple of 256 B -> the host expands
    the atom table to one 256 B slot per atom ([x,y,z,r] + pad); the
    device extracts components with stride-64 APs (free on DVE).
  - indices are int16 -> atoms are split into 4 chunks of 25000; pairs are
    grouped by (chunk(a0), chunk(a1)) into 16 groups on the host, each
    group padded to a fixed per-group call capacity so the instruction
    stream is static across cores.
  - ring capacity allows ~57 descriptors in flight per call -> 896
    indices (7 output column-groups of 128) per call.

Pipeline per 8-call block (ping-pong buffered, 5 engines):
  SP (sync):   HBM loads of per-call index tiles + packed mask bytes
  Pool:        2x8 dma_gather calls (endpoint 0 and 1)
  DVE:         dx,dy,dz,rsum; ss = |d|^2; base = rsum - dist;
               per class c: relu(base+tol_c) * (mask & 2^c), fused
               multiply+reduce accumulated into acc[128, 6]
  ACT:         dist = sqrt(ss + eps)
Host folds the 128 partials per class, the 2^-c mask scale, and the
exp(weight) factor, then sums the 8 per-core partial energies.
"""

import sys

sys.path.insert(0, "/opt/trn_rl_repo")

import numpy as np
from contextlib import ExitStack

import concourse.bacc as bacc
import concourse.bass as bass
import concourse.mybir as mybir
from concourse.library_config import mlp
from concourse.bass_utils import run_bass_kernel_spmd

F32 = mybir.dt.float32
I16 = mybir.dt.int16
U8 = mybir.dt.uint8

N_CORES = 8
EPS = 1e-12

N_ATOMS = 100000
N_PAIRS = 4000000
N_CLASS = 6

PAIRS_PER_CORE = N_PAIRS // N_CORES  # 500000

N_CHUNK = 4
CHUNK = N_ATOMS // N_CHUNK  # 25000 (< int16 max)
N_GROUPS = N_CHUNK * N_CHUNK  # 16

CALL_IDXS = 896          # indices per dma_gather call (57 descs/ring <= ~128)
CALL_COLS = CALL_IDXS // 128   # 7 output column-groups per call
CALL_IDXW = CALL_IDXS // 16    # 56 idx columns per call (wrapped-16 layout)

G_CAP_CALLS = 37         # calls per group per endpoint (fixed across cores)
G_CAP = G_CAP_CALLS * CALL_IDXS  # 33152 pair slots per group
CALLS_EP = N_GROUPS * G_CAP_CALLS  # 592 calls per endpoint
S_TOT = CALLS_EP * CALL_IDXS       # 530432 pair slots per core

BLK = 8                  # calls per compute block
NBLK = CALLS_EP // BLK   # 74
assert CALLS_EP % BLK == 0

ELEM = 64                # floats per gathered element (256 B)


def build_nc(num_devices=N_CORES, detect_races=True, variant="full"):
    do_gather = variant in ("full", "gather_only")
    do_compute = variant in ("full", "compute_only")
    nc = bacc.Bacc(
        "TRN2",
        target_bir_lowering=False,
        debug=False,
        num_devices=num_devices,
        detect_race_conditions=detect_races,
    )
    # NOTE: dma_gather ignores AP row offsets on its HBM source (verified on
    # HW), so each 25000-atom chunk must be its own tensor at offset 0.
    tables = [
        nc.dram_tensor(f"table{i}", [CHUNK, ELEM], F32, kind="ExternalInput")
        for i in range(N_CHUNK)
    ]
    idx0_h = nc.dram_tensor("idx0", [128, CALLS_EP * CALL_IDXW], I16, kind="ExternalInput")
    idx1_h = nc.dram_tensor("idx1", [128, CALLS_EP * CALL_IDXW], I16, kind="ExternalInput")
    # Masks ship as 6 separate 0/1 planes in per-block-contiguous layout.
    # (A device-side `mask & (1<<c)` would need a DVE u8 write; DVE u8 writes
    # have a sparse read-before-write-visible hazard on this HW — the next
    # DVE instruction reads ~0.1% stale elements.  DMA-written u8 that DVE
    # only READS is safe, so the bit separation is done on the host.)
    mk_h = nc.dram_tensor(
        "mk", [128, CALLS_EP // BLK, N_CLASS, BLK * CALL_COLS], U8,
        kind="ExternalInput",
    )
    toll_h = nc.dram_tensor("toll", [128, N_CLASS], F32, kind="ExternalInput")
    outp = nc.dram_tensor("out", [128, N_CLASS], F32, kind="ExternalOutput")
    tolout = nc.dram_tensor("tolout", [128, N_CLASS], F32, kind="ExternalOutput")

    NCOL = BLK * CALL_COLS  # 56 pair columns per compute block

    with ExitStack() as stack:
        ec = stack.enter_context
        block = ec(nc.Block())
        mkt = ec(nc.sbuf_tensor("mkt", [128, 2, N_CLASS, BLK * CALL_COLS], U8))
        guard0 = ec(nc.sbuf_tensor("guard0", [128, 1024], U8))
        g0t = ec(nc.sbuf_tensor("g0t", [128, 2, BLK, CALL_COLS, ELEM], F32))
        g1t = ec(nc.sbuf_tensor("g1t", [128, 2, BLK, CALL_COLS, ELEM], F32))
        idx0t = ec(nc.sbuf_tensor("idx0t", [128, 2, BLK * CALL_IDXW], I16))
        idx1t = ec(nc.sbuf_tensor("idx1t", [128, 2, BLK * CALL_IDXW], I16))
        tolb = ec(nc.sbuf_tensor("tolb", [128, N_CLASS], F32))
        # Wide per-class accumulators.  DVE writes narrower than a full
        # [128, NCOL] f32 tile (u8 tiles, [128,1] reduce outputs) are
        # sporadically not yet visible to the next DVE instruction on this
        # HW, so all block-to-block accumulation stays wide; the [128,1]
        # reductions happen once at the end and are read only by the
        # sem-gated output DMA.
        acc6 = ec(nc.sbuf_tensor("acc6", [128, N_CLASS, NCOL], F32))
        acc = ec(nc.sbuf_tensor("acc", [128, N_CLASS], F32))
        dxb = ec(nc.sbuf_tensor("dxb", [128, NCOL], F32))
        dyb = ec(nc.sbuf_tensor("dyb", [128, NCOL], F32))
        dzb = ec(nc.sbuf_tensor("dzb", [128, NCOL], F32))
        rsb = ec(nc.sbuf_tensor("rsb", [128, NCOL], F32))
        t2y = ec(nc.sbuf_tensor("t2y", [128, NCOL], F32))
        t2z = ec(nc.sbuf_tensor("t2z", [128, NCOL], F32))
        prb = ec(nc.sbuf_tensor("prb", [128, 8], F32))
        ssb = ec(nc.sbuf_tensor("ssb", [128, 2, NCOL], F32))
        distb = ec(nc.sbuf_tensor("distb", [128, 2, NCOL], F32))
        baseb = ec(nc.sbuf_tensor("baseb", [128, NCOL], F32))
        rcb = ec(nc.sbuf_tensor("rcb", [128, NCOL], F32))
        scrb = ec(nc.sbuf_tensor("scrb", [128, NCOL], F32))
        epsb = ec(nc.sbuf_tensor("epsb", [128, 1], F32))
        tol_sem = ec(nc.semaphore("tol_sem"))
        out_sem = ec(nc.semaphore("out_sem"))
        eps_sem = ec(nc.semaphore("eps_sem"))
        fin_sem = ec(nc.semaphore("fin_sem"))
        dve_sem = ec(nc.semaphore("dve_sem"))
        g0_sem = [stack.enter_context(nc.semaphore(f"g0s{b}")) for b in range(2)]
        g1_sem = [stack.enter_context(nc.semaphore(f"g1s{b}")) for b in range(2)]
        idx_sem = [stack.enter_context(nc.semaphore(f"ixs{b}")) for b in range(2)]
        mk_sem = [stack.enter_context(nc.semaphore(f"mks{b}")) for b in range(2)]
        cmp_sem = [stack.enter_context(nc.semaphore(f"cps{b}")) for b in range(2)]
        ss_sem = [stack.enter_context(nc.semaphore(f"sss{b}")) for b in range(2)]
        dist_sem = [stack.enter_context(nc.semaphore(f"dss{b}")) for b in range(2)]

        def chunks_of(call_id):
            grp = call_id // G_CAP_CALLS
            return grp // N_CHUNK, grp % N_CHUNK

        @block.sync
        def _(g):
            g.dma_start(tolb[:], toll_h[:]).then_inc(tol_sem, 16)
            for k in range(NBLK):
                b, r = k % 2, k // 2
                if k >= 2:
                    # idx tiles b consumed once round r-1's gathers completed
                    g.wait_ge(g0_sem[b], 16 * BLK * r)
                    g.wait_ge(g1_sem[b], 16 * BLK * r)
                    # mask tile b consumed once round r-1's compute completed
                    g.wait_ge(cmp_sem[b], r)
                w = BLK * CALL_IDXW
                g.dma_start(idx0t[:, b], idx0_h[:, k * w : (k + 1) * w]).then_inc(
                    idx_sem[b], 16
                )
                g.dma_start(idx1t[:, b], idx1_h[:, k * w : (k + 1) * w]).then_inc(
                    idx_sem[b], 16
                )
                g.dma_start(mkt[:, b], mk_h[:, k]).then_inc(mk_sem[b], 16)
            g.wait_ge(fin_sem, 1)
            g.dma_start(outp[:], acc[:]).then_inc(out_sem, 16)
            g.dma_start(tolout[:], tolb[:]).then_inc(out_sem, 16)
            g.wait_ge(out_sem, 32)

        @block.gpsimd
        def _(g: bass.BassGpSimd):
            g.load_library(mlp)
            for k in range(NBLK):
                b, r = k % 2, k // 2
                g.wait_ge(idx_sem[b], 32 * (r + 1))
                if k >= 2:
                    g.wait_ge(cmp_sem[b], r)  # g tiles b free
                for j in range(BLK):
                    cid = k * BLK + j
                    c0, c1 = chunks_of(cid)
                    if not do_gather:
                        g.engine_nop().then_inc(g0_sem[b], 16)
                        g.engine_nop().then_inc(g1_sem[b], 16)
                        continue
                    g.dma_gather(
                        g0t[:, b, j],
                        tables[c0][:],
                        idx0t[:, b, j * CALL_IDXW : (j + 1) * CALL_IDXW],
                        CALL_IDXS,
                        CALL_IDXS,
                        ELEM,
                    ).then_inc(g0_sem[b], 16)
                    g.dma_gather(
                        g1t[:, b, j],
                        tables[c1][:],
                        idx1t[:, b, j * CALL_IDXW : (j + 1) * CALL_IDXW],
                        CALL_IDXS,
                        CALL_IDXS,
                        ELEM,
                    ).then_inc(g1_sem[b], 16)

        @block.scalar
        def _(g):
            g.wait_ge(eps_sem, 1)
            for k in range(NBLK):
                b, r = k % 2, k // 2
                g.wait_ge(ss_sem[b], r + 1)
                if not do_compute:
                    g.activation(
                        out=distb[:, b, 0:1],
                        in_=epsb[:],
                        func=mybir.ActivationFunctionType.Sqrt,
                        bias=epsb[:],
                    ).then_inc(dist_sem[b], 1)
                    continue
                g.activation(
                    out=distb[:, b],
                    in_=ssb[:, b],
                    func=mybir.ActivationFunctionType.Sqrt,
                    bias=epsb[:],
                ).then_inc(dist_sem[b], 1)

        @block.vector
        def _(g):
            # TRN2 sequencers pipeline instruction issue: a DVE instruction
            # can read an operand before the PREVIOUS DVE instruction's write
            # to it has landed.  Tile guards every same-engine RAW/WAR with a
            # per-engine self-semaphore (each op incs it at completion;
            # dependents wait for the producer's count).  Emulate that here.
            state = {"n": 0, "w": {}, "rd": {}}

            def V(ins, reads=(), writes=()):
                ins.then_inc(dve_sem, 1)
                state["n"] += 1
                n = state["n"]
                for t in reads:
                    state["rd"][t] = n
                for t in writes:
                    state["w"][t] = n
                return ins

            def W(reads=(), writes=()):
                # wait for same-engine producers of `reads` and readers/
                # writers of `writes` to complete
                need = 0
                for t in reads:
                    need = max(need, state["w"].get(t, 0))
                for t in writes:
                    need = max(need, state["w"].get(t, 0), state["rd"].get(t, 0))
                if need > 0:
                    g.wait_ge(dve_sem, need)

            V(g.memset(acc6[:], 0.0), writes=[f"a6_{c}" for c in range(N_CLASS)])
            g.memset(epsb[:], EPS).then_inc(eps_sem, 1)  # untracked: ACT-only
            g.wait_ge(tol_sem, 16)
            for k in range(NBLK):
                b, r = k % 2, k // 2
                g.wait_ge(g0_sem[b], 16 * BLK * (r + 1))
                g.wait_ge(g1_sem[b], 16 * BLK * (r + 1))
                g.wait_ge(mk_sem[b], 16 * (r + 1))
                if not do_compute:
                    g.engine_nop().then_inc(ss_sem[b], 1)
                    g.wait_ge(dist_sem[b], r + 1)
                    g.engine_nop().then_inc(cmp_sem[b], 1)
                    continue
                G0 = g0t[:, b].rearrange("p a b c -> p (a b) c")
                G1 = g1t[:, b].rearrange("p a b c -> p (a b) c")
                ssk = f"ss{b}"
                W(writes=["dx"])
                V(g.tensor_sub(out=dxb[:], in0=G0[:, :, 0], in1=G1[:, :, 0]),
                  writes=["dx"])
                W(writes=["dy"])
                V(g.tensor_sub(out=dyb[:], in0=G0[:, :, 1], in1=G1[:, :, 1]),
                  writes=["dy"])
                W(writes=["dz"])
                V(g.tensor_sub(out=dzb[:], in0=G0[:, :, 2], in1=G1[:, :, 2]),
                  writes=["dz"])
                W(writes=["rs"])
                V(g.tensor_add(out=rsb[:], in0=G0[:, :, 3], in1=G1[:, :, 3]),
                  writes=["rs"])
                W(reads=["dx"], writes=[ssk])
                V(g.tensor_mul(out=ssb[:, b], in0=dxb[:], in1=dxb[:]),
                  reads=["dx"], writes=[ssk])
                W(reads=["dy"], writes=["t2y"])
                V(g.tensor_mul(out=t2y[:], in0=dyb[:], in1=dyb[:]),
                  reads=["dy"], writes=["t2y"])
                W(reads=["dz"], writes=["t2z"])
                V(g.tensor_mul(out=t2z[:], in0=dzb[:], in1=dzb[:]),
                  reads=["dz"], writes=["t2z"])
                W(reads=[ssk, "t2y"], writes=[ssk])
                V(g.tensor_add(out=ssb[:, b], in0=ssb[:, b], in1=t2y[:]),
                  reads=["t2y"], writes=[ssk])
                W(reads=[ssk, "t2z"], writes=[ssk])
                # carries ss_sem for ACT (single-update limit); untracked by
                # dve_sem — the next same-parity writer of ssb[b] is two
                # blocks later, transitively ordered via dist_sem.
                g.tensor_add(out=ssb[:, b], in0=ssb[:, b], in1=t2z[:]).then_inc(
                    ss_sem[b], 1
                )
                g.wait_ge(dist_sem[b], r + 1)
                W(reads=["rs"], writes=["base"])
                V(g.tensor_sub(out=baseb[:], in0=rsb[:], in1=distb[:, b]),
                  reads=["rs"], writes=["base"])
                for c in range(N_CLASS):
                    W(reads=["base"], writes=["rc"])
                    V(g.tensor_scalar(
                        out=rcb[:],
                        in0=baseb[:],
                        scalar1=tolb[:, c : c + 1],
                        scalar2=0.0,
                        op0=mybir.AluOpType.add,
                        op1=mybir.AluOpType.max,
                    ), reads=["base"], writes=["rc"])
                    W(reads=["rc"], writes=["scr"])
                    V(g.tensor_tensor(
                        out=scrb[:],
                        in0=rcb[:],
                        in1=mkt[:, b, c],
                        op=mybir.AluOpType.mult,
                    ), reads=["rc"], writes=["scr"])
                    W(reads=["scr", f"a6_{c}"], writes=[f"a6_{c}"])
                    V(g.tensor_add(out=acc6[:, c], in0=acc6[:, c], in1=scrb[:]),
                      reads=["scr"], writes=[f"a6_{c}"])
                # probe: waits for every tracked op so far, then signals the
                # block's buffers free (cmp_sem) on its own completion.
                g.wait_ge(dve_sem, state["n"])
                g.memset(prb[:], 0.0).then_inc(cmp_sem[b], 1)
            for c in range(N_CLASS):
                W(reads=[f"a6_{c}"], writes=["acc"])
                V(g.tensor_reduce(
                    out=acc[:, c : c + 1],
                    in_=acc6[:, c],
                    axis=mybir.AxisListType.X,
                    op=mybir.AluOpType.add,
                ), writes=["acc"])
            g.wait_ge(dve_sem, state["n"])
            g.memset(prb[:], 0.0).then_inc(fin_sem, 1)

    nc.compile()
    return nc


_NC_CACHE = {}


def _get_nc():
    if "nc" not in _NC_CACHE:
        _NC_CACHE["nc"] = build_nc()
    return _NC_CACHE["nc"]


def _prep_core(a0, a1, masks6):
    """Sort one core's pairs into (chunk0, chunk1) groups, pad to fixed
    per-group capacity, and emit device layouts."""
    n = a0.shape[0]
    grp = (a0 // CHUNK) * N_CHUNK + (a1 // CHUNK)
    order = np.argsort(grp, kind="stable")
    counts = np.bincount(grp, minlength=N_GROUPS)
    if counts.max() > G_CAP:
        raise RuntimeError(
            f"group capacity exceeded: max count {counts.max()} > {G_CAP}"
        )
    cum = np.zeros(N_GROUPS, dtype=np.int64)
    cum[1:] = np.cumsum(counts)[:-1]
    # position of each sorted pair within its group
    pos = np.arange(n, dtype=np.int64) - np.repeat(cum, counts)
    slot = grp[order] * np.int64(G_CAP) + pos

    idx0 = np.zeros(S_TOT, dtype=np.int16)
    idx1 = np.zeros(S_TOT, dtype=np.int16)
    idx0[slot] = (a0[order] % CHUNK).astype(np.int16)
    idx1[slot] = (a1[order] % CHUNK).astype(np.int16)

    mk = np.zeros((N_CLASS, S_TOT), dtype=np.uint8)
    mk[:, slot] = masks6[:, order].astype(np.uint8)

    def wrap16(x):
        # slot s = call*896 + col*16 + row  ->  [16, CALLS_EP*56], replicated x8
        w = (
            x.reshape(CALLS_EP, CALL_IDXW, 16)
            .transpose(2, 0, 1)
            .reshape(16, CALLS_EP * CALL_IDXW)
        )
        return np.ascontiguousarray(np.tile(w, (8, 1)))

    # mask planes in per-block layout: [128, NBLK, 6, BLK*CALL_COLS];
    # within a call, slot r -> partition r%128, column-group r//128
    mk_dev = np.ascontiguousarray(
        mk.reshape(N_CLASS, NBLK, BLK, CALL_COLS, 128)
        .transpose(4, 1, 0, 2, 3)
        .reshape(128, NBLK, N_CLASS, BLK * CALL_COLS)
    )
    return {"idx0": wrap16(idx0), "idx1": wrap16(idx1), "mk": mk_dev}


def _prep_inputs(coords, radii, tollerances, weight, atom_names, atom_pairs, clash_masks):
    """Host-side shard/layout prep. Returns (in_maps, exp_weight)."""
    coords = np.asarray(coords, dtype=np.float32)
    radii = np.asarray(radii, dtype=np.float32)
    tollerances = np.asarray(tollerances, dtype=np.float32)
    atom_names = np.asarray(atom_names).astype(np.int64)
    atom_pairs = np.asarray(atom_pairs).astype(np.int64)
    clash_masks = np.asarray(clash_masks)

    table = np.zeros((N_ATOMS, ELEM), dtype=np.float32)
    table[:, :3] = coords
    table[:, 3] = radii[atom_names]
    tchunks = [
        np.ascontiguousarray(table[i * CHUNK : (i + 1) * CHUNK])
        for i in range(N_CHUNK)
    ]

    toll2d = np.ascontiguousarray(
        np.broadcast_to(tollerances.reshape(1, N_CLASS), (128, N_CLASS))
    )

    in_maps = []
    for c in range(N_CORES):
        lo, hi = c * PAIRS_PER_CORE, (c + 1) * PAIRS_PER_CORE
        m = _prep_core(
            atom_pairs[lo:hi, 0], atom_pairs[lo:hi, 1], clash_masks[:, lo:hi]
        )
        for i in range(N_CHUNK):
            m[f"table{i}"] = tchunks[i]
        m["toll"] = toll2d
        in_maps.append(m)
    return in_maps, float(np.exp(np.float64(np.asarray(weight).reshape(-1)[0])))


def _finalize(outs, wscale):
    """outs: list of per-core [128, 6] partials. Fold partitions and the
    exp(weight) scale."""
    total = np.zeros(N_CLASS, dtype=np.float64)
    for o in outs:
        total += np.asarray(o, dtype=np.float64).reshape(128, N_CLASS).sum(axis=0)
    return (total * wscale).astype(np.float32)


def kernel(coords, radii, tollerances, weight, atom_names, atom_pairs, clash_masks):
    nc = _get_nc()
    in_maps, wscale = _prep_inputs(
        coords, radii, tollerances, weight, atom_names, atom_pairs, clash_masks
    )
    res = run_bass_kernel_spmd(nc, in_maps, core_ids=list(range(N_CORES)))
    return _finalize([res.results[c]["out"] for c in range(N_CORES)], wscale)


# revision 43
# speedup vs baseline: 1.1869x; 1.1869x over previous
"""Trainium2 Bass kernel for nn_Clash_net (clash energy over atom pairs).

Contract: kernel(**inputs) takes FULL (unsharded) numpy inputs as produced by
setup_inputs() and returns the FULL [6] float32 energies output.

Strategy (8 NeuronCores, SPMD over the pair dimension):

The problem is gather-bound: each of the 4M pairs needs two random 16 B
records ([x,y,z,r]) from a 100K-atom table.  The previous kernel used
gpsimd.indirect_dma_start with [128,1] offsets (~24 us per 128 records,
Q7 cross-partition offset reads) -> ~95 ms.  This kernel replaces it with
the vectorized `dma_gather` custom instruction (mlp ucode library): ONE
instruction gathers 896 records by an int16 index list that the Q7 cores
read from their own 16 partitions, emitting descriptors 16-at-a-time with
SIMD pushes.  Measured throughput is ~100x the indirect-DMA path.

dma_gather constraints and how they are met:
  - gathered element size must be a multiple of 256 B -> the host expands
    the atom table to one 256 B slot per atom ([x,y,z,r] + pad); the
    device extracts components with stride-64 APs (free on DVE).
  - indices are int16 -> atoms are split into 4 chunks of 25000; pairs are
    grouped by (chunk(a0), chunk(a1)) into 16 groups on the host, each
    group padded to a fixed per-group call capacity so the instruction
    stream is static across cores.
  - ring capacity allows ~57 descriptors in flight per call -> 896
    indices (7 output column-groups of 128) per call.

Pipeline per 8-call block (ping-pong buffered, 5 engines):
  SP (sync):   HBM loads of per-call index tiles + packed mask bytes
  Pool:        2x8 dma_gather calls (endpoint 0 and 1)
  DVE:         dx,dy,dz,rsum; ss = |d|^2; base = rsum - dist;
               per class c: relu(base+tol_c) * (mask & 2^c), fused
               multiply+reduce accumulated into acc[128, 6]
  ACT:         dist = sqrt(ss + eps)
Host folds the 128 partials per class, the 2^-c mask scale, and the
exp(weight) factor, then sums the 8 per-core partial energies.
"""

import sys

sys.path.insert(0, "/opt/trn_rl_repo")

import numpy as np
from contextlib import ExitStack

import concourse.bacc as bacc
import concourse.bass as bass
import concourse.mybir as mybir
from concourse.library_config import mlp
from concourse.bass_utils import run_bass_kernel_spmd

F32 = mybir.dt.float32
I16 = mybir.dt.int16
U8 = mybir.dt.uint8

N_CORES = 8
EPS = 1e-12

N_ATOMS = 100000
N_PAIRS = 4000000
N_CLASS = 6

PAIRS_PER_CORE = N_PAIRS // N_CORES  # 500000

N_CHUNK = 4
CHUNK = N_ATOMS // N_CHUNK  # 25000 (< int16 max)
N_GROUPS = N_CHUNK * N_CHUNK  # 16

CALL_IDXS = 896          # indices per dma_gather call (57 descs/ring <= ~128)
CALL_COLS = CALL_IDXS // 128   # 7 output column-groups per call
CALL_IDXW = CALL_IDXS // 16    # 56 idx columns per call (wrapped-16 layout)

G_CAP_CALLS = 37         # calls per group per endpoint (fixed across cores)
G_CAP = G_CAP_CALLS * CALL_IDXS  # 33152 pair slots per group
CALLS_EP = N_GROUPS * G_CAP_CALLS  # 592 calls per endpoint
S_TOT = CALLS_EP * CALL_IDXS       # 530432 pair slots per core

BLK = 8                  # calls per compute block
NBLK = CALLS_EP // BLK   # 74
assert CALLS_EP % BLK == 0

ELEM = 64                # floats per gathered element (256 B)


def build_nc(num_devices=N_CORES, detect_races=True, variant="full"):
    do_gather = variant in ("full", "gather_only")
    do_compute = variant in ("full", "compute_only")
    nc = bacc.Bacc(
        "TRN2",
        target_bir_lowering=False,
        debug=False,
        num_devices=num_devices,
        detect_race_conditions=detect_races,
    )
    # Per-call wall time through the axon tunnel scales with input bytes, so
    # inputs are shipped compact: a [N_ATOMS, 4] table expanded on-device into
    # Internal DRAM (dma_gather needs 256 B elements and ignores source row
    # offsets, so each 25000-atom chunk is its own Internal tensor at offset
    # 0), idx lists replicated only 2x (queue-0 dma_gather reads them from
    # partitions 0-31 only), and masks packed 6-bits-per-pair.
    tbl_h = nc.dram_tensor("tbl", [N_ATOMS, 4], F32, kind="ExternalInput")
    tables = [
        nc.dram_tensor(f"tbx{i}", [CHUNK, ELEM], F32, kind="Internal")
        for i in range(N_CHUNK)
    ]
    idx0_h = nc.dram_tensor("idx0", [32, CALLS_EP * CALL_IDXW], I16, kind="ExternalInput")
    idx1_h = nc.dram_tensor("idx1", [32, CALLS_EP * CALL_IDXW], I16, kind="ExternalInput")
    mk_h = nc.dram_tensor(
        "mk", [128, CALLS_EP // BLK, BLK * CALL_COLS], U8, kind="ExternalInput"
    )
    toll_h = nc.dram_tensor("toll", [128, N_CLASS], F32, kind="ExternalInput")
    outp = nc.dram_tensor("out", [128, N_CLASS], F32, kind="ExternalOutput")

    NCOL = BLK * CALL_COLS  # 56 pair columns per compute block

    with ExitStack() as stack:
        ec = stack.enter_context
        block = ec(nc.Block())
        mkt = ec(nc.sbuf_tensor("mkt", [128, 2, BLK * CALL_COLS], U8))
        g0t = ec(nc.sbuf_tensor("g0t", [128, 2, BLK, CALL_COLS, ELEM], F32))
        g1t = ec(nc.sbuf_tensor("g1t", [128, 2, BLK, CALL_COLS, ELEM], F32))
        idx0t = ec(nc.sbuf_tensor("idx0t", [32, 2, BLK * CALL_IDXW], I16))
        idx1t = ec(nc.sbuf_tensor("idx1t", [32, 2, BLK * CALL_IDXW], I16))
        tolb = ec(nc.sbuf_tensor("tolb", [128, N_CLASS], F32))
        # Wide per-class accumulators.  DVE writes narrower than a full
        # [128, NCOL] f32 tile (u8 tiles, [128,1] reduce outputs) are
        # sporadically not yet visible to the next DVE instruction on this
        # HW, so all block-to-block accumulation stays wide; the [128,1]
        # reductions happen once at the end and are read only by the
        # sem-gated output DMA.
        acc6 = ec(nc.sbuf_tensor("acc6", [128, N_CLASS, NCOL], F32))
        acc = ec(nc.sbuf_tensor("acc", [128, N_CLASS], F32))
        dxb = ec(nc.sbuf_tensor("dxb", [128, NCOL], F32))
        dyb = ec(nc.sbuf_tensor("dyb", [128, NCOL], F32))
        dzb = ec(nc.sbuf_tensor("dzb", [128, NCOL], F32))
        rsb = ec(nc.sbuf_tensor("rsb", [128, NCOL], F32))
        t2y = ec(nc.sbuf_tensor("t2y", [128, NCOL], F32))
        t2z = ec(nc.sbuf_tensor("t2z", [128, NCOL], F32))
        prb = ec(nc.sbuf_tensor("prb", [128, 8], F32))
        ssb = ec(nc.sbuf_tensor("ssb", [128, 2, NCOL], F32))
        distb = ec(nc.sbuf_tensor("distb", [128, 2, NCOL], F32))
        baseb = ec(nc.sbuf_tensor("baseb", [128, NCOL], F32))
        rcb = ec(nc.sbuf_tensor("rcb", [128, NCOL], F32))
        mcb = ec(nc.sbuf_tensor("mcb", [128, NCOL], U8))
        scrb = ec(nc.sbuf_tensor("scrb", [128, NCOL], F32))
        epsb = ec(nc.sbuf_tensor("epsb", [128, 1], F32))
        tol_sem = ec(nc.semaphore("tol_sem"))
        out_sem = ec(nc.semaphore("out_sem"))
        eps_sem = ec(nc.semaphore("eps_sem"))
        fin_sem = ec(nc.semaphore("fin_sem"))
        dve_sem = ec(nc.semaphore("dve_sem"))
        tbl_sem = ec(nc.semaphore("tbl_sem"))
        g0_sem = [stack.enter_context(nc.semaphore(f"g0s{b}")) for b in range(2)]
        g1_sem = [stack.enter_context(nc.semaphore(f"g1s{b}")) for b in range(2)]
        idx_sem = [stack.enter_context(nc.semaphore(f"ixs{b}")) for b in range(2)]
        mk_sem = [stack.enter_context(nc.semaphore(f"mks{b}")) for b in range(2)]
        cmp_sem = [stack.enter_context(nc.semaphore(f"cps{b}")) for b in range(2)]
        ss_sem = [stack.enter_context(nc.semaphore(f"sss{b}")) for b in range(2)]
        dist_sem = [stack.enter_context(nc.semaphore(f"dss{b}")) for b in range(2)]

        def chunks_of(call_id):
            grp = call_id // G_CAP_CALLS
            return grp // N_CHUNK, grp % N_CHUNK

        @block.sync
        def _(g):
            for i in range(N_CHUNK):
                g.dma_start(
                    tables[i][:, 0:4], tbl_h[i * CHUNK : (i + 1) * CHUNK, :]
                ).then_inc(tbl_sem, 16)
            g.dma_start(tolb[:], toll_h[:]).then_inc(tol_sem, 16)
            for k in range(NBLK):
                b, r = k % 2, k // 2
                if k >= 2:
                    # idx tiles b consumed once round r-1's gathers completed
                    g.wait_ge(g0_sem[b], 16 * BLK * r)
                    g.wait_ge(g1_sem[b], 16 * BLK * r)
                    # mask tile b consumed once round r-1's compute completed
                    g.wait_ge(cmp_sem[b], r)
                w = BLK * CALL_IDXW
                g.dma_start(idx0t[:, b], idx0_h[:, k * w : (k + 1) * w]).then_inc(
                    idx_sem[b], 16
                )
                g.dma_start(idx1t[:, b], idx1_h[:, k * w : (k + 1) * w]).then_inc(
                    idx_sem[b], 16
                )
                g.dma_start(mkt[:, b], mk_h[:, k]).then_inc(mk_sem[b], 16)
            g.wait_ge(fin_sem, 1)
            g.dma_start(outp[:], acc[:]).then_inc(out_sem, 16)
            g.wait_ge(out_sem, 16)

        @block.gpsimd
        def _(g: bass.BassGpSimd):
            g.load_library(mlp)
            g.wait_ge(tbl_sem, 16 * N_CHUNK)
            for k in range(NBLK):
                b, r = k % 2, k // 2
                g.wait_ge(idx_sem[b], 32 * (r + 1))
                if k >= 2:
                    g.wait_ge(cmp_sem[b], r)  # g tiles b free
                for j in range(BLK):
                    cid = k * BLK + j
                    c0, c1 = chunks_of(cid)
                    if not do_gather:
                        g.engine_nop().then_inc(g0_sem[b], 16)
                        g.engine_nop().then_inc(g1_sem[b], 16)
                        continue
                    g.dma_gather(
                        g0t[:, b, j],
                        tables[c0][:],
                        idx0t[:, b, j * CALL_IDXW : (j + 1) * CALL_IDXW],
                        CALL_IDXS,
                        CALL_IDXS,
                        ELEM,
                    ).then_inc(g0_sem[b], 16)
                    g.dma_gather(
                        g1t[:, b, j],
                        tables[c1][:],
                        idx1t[:, b, j * CALL_IDXW : (j + 1) * CALL_IDXW],
                        CALL_IDXS,
                        CALL_IDXS,
                        ELEM,
                    ).then_inc(g1_sem[b], 16)

        @block.scalar
        def _(g):
            g.wait_ge(eps_sem, 1)
            for k in range(NBLK):
                b, r = k % 2, k // 2
                g.wait_ge(ss_sem[b], r + 1)
                if not do_compute:
                    g.activation(
                        out=distb[:, b, 0:1],
                        in_=epsb[:],
                        func=mybir.ActivationFunctionType.Sqrt,
                        bias=epsb[:],
                    ).then_inc(dist_sem[b], 1)
                    continue
                g.activation(
                    out=distb[:, b],
                    in_=ssb[:, b],
                    func=mybir.ActivationFunctionType.Sqrt,
                    bias=epsb[:],
                ).then_inc(dist_sem[b], 1)

        @block.vector
        def _(g):
            # TRN2 sequencers pipeline instruction issue: a DVE instruction
            # can read an operand before the PREVIOUS DVE instruction's write
            # to it has landed.  Tile guards every same-engine RAW/WAR with a
            # per-engine self-semaphore (each op incs it at completion;
            # dependents wait for the producer's count).  Emulate that here.
            state = {"n": 0, "w": {}, "rd": {}}

            def V(ins, reads=(), writes=()):
                ins.then_inc(dve_sem, 1)
                state["n"] += 1
                n = state["n"]
                for t in reads:
                    state["rd"][t] = n
                for t in writes:
                    state["w"][t] = n
                return ins

            def W(reads=(), writes=()):
                # wait for same-engine producers of `reads` and readers/
                # writers of `writes` to complete
                need = 0
                for t in reads:
                    need = max(need, state["w"].get(t, 0))
                for t in writes:
                    need = max(need, state["w"].get(t, 0), state["rd"].get(t, 0))
                if need > 0:
                    g.wait_ge(dve_sem, need)

            V(g.memset(acc6[:], 0.0), writes=[f"a6_{c}" for c in range(N_CLASS)])
            g.memset(epsb[:], EPS).then_inc(eps_sem, 1)  # untracked: ACT-only
            g.wait_ge(tol_sem, 16)
            for k in range(NBLK):
                b, r = k % 2, k // 2
                g.wait_ge(g0_sem[b], 16 * BLK * (r + 1))
                g.wait_ge(g1_sem[b], 16 * BLK * (r + 1))
                g.wait_ge(mk_sem[b], 16 * (r + 1))
                if not do_compute:
                    g.engine_nop().then_inc(ss_sem[b], 1)
                    g.wait_ge(dist_sem[b], r + 1)
                    g.engine_nop().then_inc(cmp_sem[b], 1)
                    continue
                G0 = g0t[:, b].rearrange("p a b c -> p (a b) c")
                G1 = g1t[:, b].rearrange("p a b c -> p (a b) c")
                ssk = f"ss{b}"
                W(writes=["dx"])
                V(g.tensor_sub(out=dxb[:], in0=G0[:, :, 0], in1=G1[:, :, 0]),
                  writes=["dx"])
                W(writes=["dy"])
                V(g.tensor_sub(out=dyb[:], in0=G0[:, :, 1], in1=G1[:, :, 1]),
                  writes=["dy"])
                W(writes=["dz"])
                V(g.tensor_sub(out=dzb[:], in0=G0[:, :, 2], in1=G1[:, :, 2]),
                  writes=["dz"])
                W(writes=["rs"])
                V(g.tensor_add(out=rsb[:], in0=G0[:, :, 3], in1=G1[:, :, 3]),
                  writes=["rs"])
                W(reads=["dx"], writes=[ssk])
                V(g.tensor_mul(out=ssb[:, b], in0=dxb[:], in1=dxb[:]),
                  reads=["dx"], writes=[ssk])
                W(reads=["dy"], writes=["t2y"])
                V(g.tensor_mul(out=t2y[:], in0=dyb[:], in1=dyb[:]),
                  reads=["dy"], writes=["t2y"])
                W(reads=["dz"], writes=["t2z"])
                V(g.tensor_mul(out=t2z[:], in0=dzb[:], in1=dzb[:]),
                  reads=["dz"], writes=["t2z"])
                W(reads=[ssk, "t2y"], writes=[ssk])
                V(g.tensor_add(out=ssb[:, b], in0=ssb[:, b], in1=t2y[:]),
                  reads=["t2y"], writes=[ssk])
                W(reads=[ssk, "t2z"], writes=[ssk])
                # carries ss_sem for ACT (single-update limit); untracked by
                # dve_sem — the next same-parity writer of ssb[b] is two
                # blocks later, transitively ordered via dist_sem.
                g.tensor_add(out=ssb[:, b], in0=ssb[:, b], in1=t2z[:]).then_inc(
                    ss_sem[b], 1
                )
                g.wait_ge(dist_sem[b], r + 1)
                W(reads=["rs"], writes=["base"])
                V(g.tensor_sub(out=baseb[:], in0=rsb[:], in1=distb[:, b]),
                  reads=["rs"], writes=["base"])
                for c in range(N_CLASS):
                    W(reads=["base"], writes=["rc"])
                    V(g.tensor_scalar(
                        out=rcb[:],
                        in0=baseb[:],
                        scalar1=tolb[:, c : c + 1],
                        scalar2=0.0,
                        op0=mybir.AluOpType.add,
                        op1=mybir.AluOpType.max,
                    ), reads=["base"], writes=["rc"])
                    W(writes=["mc"])
                    V(g.tensor_scalar(
                        out=mcb[:],
                        in0=mkt[:, b],
                        scalar1=1 << c,
                        scalar2=None,
                        op0=mybir.AluOpType.bitwise_and,
                    ), writes=["mc"])
                    W(reads=["rc", "mc"], writes=["scr"])
                    V(g.tensor_tensor(
                        out=scrb[:],
                        in0=rcb[:],
                        in1=mcb[:],
                        op=mybir.AluOpType.mult,
                    ), reads=["rc", "mc"], writes=["scr"])
                    W(reads=["scr", f"a6_{c}"], writes=[f"a6_{c}"])
                    V(g.tensor_add(out=acc6[:, c], in0=acc6[:, c], in1=scrb[:]),
                      reads=["scr"], writes=[f"a6_{c}"])
                # probe: waits for every tracked op so far, then signals the
                # block's buffers free (cmp_sem) on its own completion.
                g.wait_ge(dve_sem, state["n"])
                g.memset(prb[:], 0.0).then_inc(cmp_sem[b], 1)
            for c in range(N_CLASS):
                W(reads=[f"a6_{c}"], writes=["acc"])
                V(g.tensor_reduce(
                    out=acc[:, c : c + 1],
                    in_=acc6[:, c],
                    axis=mybir.AxisListType.X,
                    op=mybir.AluOpType.add,
                ), writes=["acc"])
            g.wait_ge(dve_sem, state["n"])
            g.memset(prb[:], 0.0).then_inc(fin_sem, 1)

    nc.compile()
    return nc


_NC_CACHE = {}


def _get_nc():
    if "nc" not in _NC_CACHE:
        _NC_CACHE["nc"] = build_nc()
    return _NC_CACHE["nc"]


def _prep_core(a0, a1, masks6):
    """Sort one core's pairs into (chunk0, chunk1) groups, pad to fixed
    per-group capacity, and emit device layouts."""
    n = a0.shape[0]
    grp = (a0 // CHUNK) * N_CHUNK + (a1 // CHUNK)
    order = np.argsort(grp, kind="stable")
    counts = np.bincount(grp, minlength=N_GROUPS)
    if counts.max() > G_CAP:
        raise RuntimeError(
            f"group capacity exceeded: max count {counts.max()} > {G_CAP}"
        )
    cum = np.zeros(N_GROUPS, dtype=np.int64)
    cum[1:] = np.cumsum(counts)[:-1]
    # position of each sorted pair within its group
    pos = np.arange(n, dtype=np.int64) - np.repeat(cum, counts)
    slot = grp[order] * np.int64(G_CAP) + pos

    idx0 = np.zeros(S_TOT, dtype=np.int16)
    idx1 = np.zeros(S_TOT, dtype=np.int16)
    idx0[slot] = (a0[order] % CHUNK).astype(np.int16)
    idx1[slot] = (a1[order] % CHUNK).astype(np.int16)

    bits = np.zeros(n, dtype=np.uint8)
    for c in range(N_CLASS):
        bits |= masks6[c, order].astype(np.uint8) << c
    mk = np.zeros(S_TOT, dtype=np.uint8)
    mk[slot] = bits

    def wrap16(x):
        # slot s = call*896 + col*16 + row -> [16, CALLS_EP*56], replicated
        # x2 (queue-0 dma_gather's tx/rx Q7 cores read partitions 0-15/16-31)
        w = (
            x.reshape(CALLS_EP, CALL_IDXW, 16)
            .transpose(2, 0, 1)
            .reshape(16, CALLS_EP * CALL_IDXW)
        )
        return np.ascontiguousarray(np.tile(w, (2, 1)))

    # packed mask bytes in per-block layout: [128, NBLK, BLK*CALL_COLS];
    # within a call, slot r -> partition r%128, column-group r//128
    mk_dev = np.ascontiguousarray(
        mk.reshape(NBLK, BLK, CALL_COLS, 128)
        .transpose(3, 0, 1, 2)
        .reshape(128, NBLK, BLK * CALL_COLS)
    )
    return {"idx0": wrap16(idx0), "idx1": wrap16(idx1), "mk": mk_dev}


def _prep_inputs(coords, radii, tollerances, weight, atom_names, atom_pairs, clash_masks):
    """Host-side shard/layout prep. Returns (in_maps, exp_weight)."""
    coords = np.asarray(coords, dtype=np.float32)
    radii = np.asarray(radii, dtype=np.float32)
    tollerances = np.asarray(tollerances, dtype=np.float32)
    atom_names = np.asarray(atom_names).astype(np.int64)
    atom_pairs = np.asarray(atom_pairs).astype(np.int64)
    clash_masks = np.asarray(clash_masks)

    table = np.zeros((N_ATOMS, 4), dtype=np.float32)
    table[:, :3] = coords
    table[:, 3] = radii[atom_names]

    toll2d = np.ascontiguousarray(
        np.broadcast_to(tollerances.reshape(1, N_CLASS), (128, N_CLASS))
    )

    in_maps = []
    for c in range(N_CORES):
        lo, hi = c * PAIRS_PER_CORE, (c + 1) * PAIRS_PER_CORE
        m = _prep_core(
            atom_pairs[lo:hi, 0], atom_pairs[lo:hi, 1], clash_masks[:, lo:hi]
        )
        m["tbl"] = table
        m["toll"] = toll2d
        in_maps.append(m)
    return in_maps, float(np.exp(np.float64(np.asarray(weight).reshape(-1)[0])))


def _finalize(outs, wscale):
    """outs: list of per-core [128, 6] partials. Fold partitions, the 2^c
    mask-bit scale, and exp(weight)."""
    total = np.zeros(N_CLASS, dtype=np.float64)
    for o in outs:
        total += np.asarray(o, dtype=np.float64).reshape(128, N_CLASS).sum(axis=0)
    total /= np.exp2(np.arange(N_CLASS, dtype=np.float64))
    return (total * wscale).astype(np.float32)


def kernel(coords, radii, tollerances, weight, atom_names, atom_pairs, clash_masks):
    nc = _get_nc()
    in_maps, wscale = _prep_inputs(
        coords, radii, tollerances, weight, atom_names, atom_pairs, clash_masks
    )
    res = run_bass_kernel_spmd(nc, in_maps, core_ids=list(range(N_CORES)))
    return _finalize([res.results[c]["out"] for c in range(N_CORES)], wscale)


# revision 44
# speedup vs baseline: 1.4434x; 1.2161x over previous
"""Trainium2 Bass kernel for nn_Clash_net (clash energy over atom pairs).

Contract: kernel(**inputs) takes FULL (unsharded) numpy inputs as produced by
setup_inputs() and returns the FULL [6] float32 energies output.

Strategy (8 NeuronCores, SPMD over the pair dimension):

The problem is gather-bound: each of the 4M pairs needs two random 16 B
records ([x,y,z,r]) from a 100K-atom table.  The previous kernel used
gpsimd.indirect_dma_start with [128,1] offsets (~24 us per 128 records,
Q7 cross-partition offset reads) -> ~95 ms.  This kernel replaces it with
the vectorized `dma_gather` custom instruction (mlp ucode library): ONE
instruction gathers 896 records by an int16 index list that the Q7 cores
read from their own 16 partitions, emitting descriptors 16-at-a-time with
SIMD pushes.  Measured throughput is ~100x the indirect-DMA path.

dma_gather constraints and how they are met:
  - gathered element size must be a multiple of 256 B -> the host expands
    the atom table to one 256 B slot per atom ([x,y,z,r] + pad); the
    device extracts components with stride-64 APs (free on DVE).
  - indices are int16 -> atoms are split into 4 chunks of 25000; pairs are
    grouped by (chunk(a0), chunk(a1)) into 16 groups on the host, each
    group padded to a fixed per-group call capacity so the instruction
    stream is static across cores.
  - ring capacity allows ~57 descriptors in flight per call -> 896
    indices (7 output column-groups of 128) per call.

Pipeline per 8-call block (ping-pong buffered, 5 engines):
  SP (sync):   HBM loads of per-call index tiles + packed mask bytes
  Pool:        2x8 dma_gather calls (endpoint 0 and 1)
  DVE:         dx,dy,dz,rsum; ss = |d|^2; base = rsum - dist;
               per class c: relu(base+tol_c) * (mask & 2^c), fused
               multiply+reduce accumulated into acc[128, 6]
  ACT:         dist = sqrt(ss + eps)
Host folds the 128 partials per class, the 2^-c mask scale, and the
exp(weight) factor, then sums the 8 per-core partial energies.
"""

import sys

sys.path.insert(0, "/opt/trn_rl_repo")

import numpy as np
from contextlib import ExitStack

import concourse.bacc as bacc
import concourse.bass as bass
import concourse.mybir as mybir
from concourse.library_config import mlp
from concourse.bass_utils import run_bass_kernel_spmd

F32 = mybir.dt.float32
I16 = mybir.dt.int16
U8 = mybir.dt.uint8

N_CORES = 8
EPS = 1e-12

N_ATOMS = 100000
N_PAIRS = 4000000
N_CLASS = 6

PAIRS_PER_CORE = N_PAIRS // N_CORES  # 500000

N_CHUNK = 4
CHUNK = N_ATOMS // N_CHUNK  # 25000 (< int16 max)
N_GROUPS = N_CHUNK * N_CHUNK  # 16

CALL_IDXS = 896          # indices per dma_gather call (57 descs/ring <= ~128)
CALL_COLS = CALL_IDXS // 128   # 7 output column-groups per call
CALL_IDXW = CALL_IDXS // 16    # 56 idx columns per call (wrapped-16 layout)

G_CAP_CALLS = 37         # calls per group per endpoint (fixed across cores)
G_CAP = G_CAP_CALLS * CALL_IDXS  # 33152 pair slots per group
CALLS_EP = N_GROUPS * G_CAP_CALLS  # 592 calls per endpoint
S_TOT = CALLS_EP * CALL_IDXS       # 530432 pair slots per core

BLK = 8                  # calls per compute block
NBLK = CALLS_EP // BLK   # 74
assert CALLS_EP % BLK == 0

ELEM = 64                # floats per gathered element (256 B)


def build_nc(num_devices=N_CORES, detect_races=True, variant="full"):
    do_gather = variant in ("full", "gather_only")
    do_compute = variant in ("full", "compute_only")
    nc = bacc.Bacc(
        "TRN2",
        target_bir_lowering=False,
        debug=False,
        num_devices=num_devices,
        detect_race_conditions=detect_races,
        num_swdge_queues=4,
    )
    # Per-call wall time through the axon tunnel scales with input bytes, so
    # inputs are shipped compact: a [N_ATOMS, 4] table expanded on-device into
    # Internal DRAM (dma_gather needs 256 B elements and ignores source row
    # offsets, so each 25000-atom chunk is its own Internal tensor at offset
    # 0), idx lists replicated only 2x (queue-0 dma_gather reads them from
    # partitions 0-31 only), and masks packed 6-bits-per-pair.
    tbl_h = nc.dram_tensor("tbl", [N_ATOMS, 4], F32, kind="ExternalInput")
    tables = [
        nc.dram_tensor(f"tbx{i}", [CHUNK, ELEM], F32, kind="Internal")
        for i in range(N_CHUNK)
    ]
    # Queue-banded idx layout: dma_gather on queue q reads its int16 index
    # list from partition rows [32q, 32q+32) (tx/rx Q7-core copies in the two
    # 16-row halves).  Calls rotate queues -- endpoint 0 on queues 0/1,
    # endpoint 1 on queues 2/3 by call parity -- so the 4 bands of one
    # [128, 4*CALL_IDXW] block tile carry 4 different calls' indices and a
    # single DMA per block loads them all.
    idx_h = nc.dram_tensor(
        "idx", [128, CALLS_EP // BLK, (BLK // 2) * CALL_IDXW], I16,
        kind="ExternalInput",
    )
    mk_h = nc.dram_tensor(
        "mk", [128, CALLS_EP // BLK, BLK * CALL_COLS], U8, kind="ExternalInput"
    )
    toll_h = nc.dram_tensor("toll", [128, N_CLASS], F32, kind="ExternalInput")
    outp = nc.dram_tensor("out", [128, N_CLASS], F32, kind="ExternalOutput")

    NCOL = BLK * CALL_COLS  # 56 pair columns per compute block

    with ExitStack() as stack:
        ec = stack.enter_context
        block = ec(nc.Block())
        mkt = ec(nc.sbuf_tensor("mkt", [128, 2, BLK * CALL_COLS], U8))
        g0t = ec(nc.sbuf_tensor("g0t", [128, 2, BLK, CALL_COLS, ELEM], F32))
        g1t = ec(nc.sbuf_tensor("g1t", [128, 2, BLK, CALL_COLS, ELEM], F32))
        idxt = ec(nc.sbuf_tensor("idxt", [128, 2, (BLK // 2) * CALL_IDXW], I16))
        tolb = ec(nc.sbuf_tensor("tolb", [128, N_CLASS], F32))
        # Wide per-class accumulators.  DVE writes narrower than a full
        # [128, NCOL] f32 tile (u8 tiles, [128,1] reduce outputs) are
        # sporadically not yet visible to the next DVE instruction on this
        # HW, so all block-to-block accumulation stays wide; the [128,1]
        # reductions happen once at the end and are read only by the
        # sem-gated output DMA.
        acc6 = ec(nc.sbuf_tensor("acc6", [128, N_CLASS, NCOL], F32))
        acc = ec(nc.sbuf_tensor("acc", [128, N_CLASS], F32))
        dxb = ec(nc.sbuf_tensor("dxb", [128, NCOL], F32))
        dyb = ec(nc.sbuf_tensor("dyb", [128, NCOL], F32))
        dzb = ec(nc.sbuf_tensor("dzb", [128, NCOL], F32))
        rsb = ec(nc.sbuf_tensor("rsb", [128, NCOL], F32))
        t2y = ec(nc.sbuf_tensor("t2y", [128, NCOL], F32))
        t2z = ec(nc.sbuf_tensor("t2z", [128, NCOL], F32))
        prb = ec(nc.sbuf_tensor("prb", [128, 8], F32))
        ssb = ec(nc.sbuf_tensor("ssb", [128, 2, NCOL], F32))
        distb = ec(nc.sbuf_tensor("distb", [128, 2, NCOL], F32))
        baseb = ec(nc.sbuf_tensor("baseb", [128, NCOL], F32))
        rcb = ec(nc.sbuf_tensor("rcb", [128, NCOL], F32))
        mcb = ec(nc.sbuf_tensor("mcb", [128, NCOL], U8))
        scrb = ec(nc.sbuf_tensor("scrb", [128, NCOL], F32))
        epsb = ec(nc.sbuf_tensor("epsb", [128, 1], F32))
        tol_sem = ec(nc.semaphore("tol_sem"))
        out_sem = ec(nc.semaphore("out_sem"))
        eps_sem = ec(nc.semaphore("eps_sem"))
        fin_sem = ec(nc.semaphore("fin_sem"))
        dve_sem = ec(nc.semaphore("dve_sem"))
        tbl_sem = ec(nc.semaphore("tbl_sem"))
        g0_sem = [stack.enter_context(nc.semaphore(f"g0s{b}")) for b in range(2)]
        g1_sem = [stack.enter_context(nc.semaphore(f"g1s{b}")) for b in range(2)]
        idx_sem = [stack.enter_context(nc.semaphore(f"ixs{b}")) for b in range(2)]
        mk_sem = [stack.enter_context(nc.semaphore(f"mks{b}")) for b in range(2)]
        cmp_sem = [stack.enter_context(nc.semaphore(f"cps{b}")) for b in range(2)]
        ss_sem = [stack.enter_context(nc.semaphore(f"sss{b}")) for b in range(2)]
        dist_sem = [stack.enter_context(nc.semaphore(f"dss{b}")) for b in range(2)]

        def chunks_of(call_id):
            grp = call_id // G_CAP_CALLS
            return grp // N_CHUNK, grp % N_CHUNK

        @block.sync
        def _(g):
            for i in range(N_CHUNK):
                g.dma_start(
                    tables[i][:, 0:4], tbl_h[i * CHUNK : (i + 1) * CHUNK, :]
                ).then_inc(tbl_sem, 16)
            g.dma_start(tolb[:], toll_h[:]).then_inc(tol_sem, 16)
            for k in range(NBLK):
                b, r = k % 2, k // 2
                if k >= 2:
                    # idx tiles b consumed once round r-1's gathers completed
                    g.wait_ge(g0_sem[b], 16 * BLK * r)
                    g.wait_ge(g1_sem[b], 16 * BLK * r)
                    # mask tile b consumed once round r-1's compute completed
                    g.wait_ge(cmp_sem[b], r)
                g.dma_start(idxt[:, b], idx_h[:, k]).then_inc(idx_sem[b], 16)
                g.dma_start(mkt[:, b], mk_h[:, k]).then_inc(mk_sem[b], 16)
            g.wait_ge(fin_sem, 1)
            g.dma_start(outp[:], acc[:]).then_inc(out_sem, 16)
            g.wait_ge(out_sem, 16)

        @block.gpsimd
        def _(g: bass.BassGpSimd):
            g.load_library(mlp)
            g.wait_ge(tbl_sem, 16 * N_CHUNK)
            for k in range(NBLK):
                b, r = k % 2, k // 2
                g.wait_ge(idx_sem[b], 16 * (r + 1))
                if k >= 2:
                    g.wait_ge(cmp_sem[b], r)  # g tiles b free
                for j in range(BLK):
                    cid = k * BLK + j
                    c0, c1 = chunks_of(cid)
                    if not do_gather:
                        g.engine_nop().then_inc(g0_sem[b], 16)
                        g.engine_nop().then_inc(g1_sem[b], 16)
                        continue
                    ixs = idxt[:, b, (j // 2) * CALL_IDXW : (j // 2 + 1) * CALL_IDXW]
                    g.dma_gather(
                        g0t[:, b, j],
                        tables[c0][:],
                        ixs,
                        CALL_IDXS,
                        CALL_IDXS,
                        ELEM,
                        queue_num=j % 2,
                    ).then_inc(g0_sem[b], 16)
                    g.dma_gather(
                        g1t[:, b, j],
                        tables[c1][:],
                        ixs,
                        CALL_IDXS,
                        CALL_IDXS,
                        ELEM,
                        queue_num=2 + (j % 2),
                    ).then_inc(g1_sem[b], 16)

        @block.scalar
        def _(g):
            g.wait_ge(eps_sem, 1)
            for k in range(NBLK):
                b, r = k % 2, k // 2
                g.wait_ge(ss_sem[b], r + 1)
                if not do_compute:
                    g.activation(
                        out=distb[:, b, 0:1],
                        in_=epsb[:],
                        func=mybir.ActivationFunctionType.Sqrt,
                        bias=epsb[:],
                    ).then_inc(dist_sem[b], 1)
                    continue
                g.activation(
                    out=distb[:, b],
                    in_=ssb[:, b],
                    func=mybir.ActivationFunctionType.Sqrt,
                    bias=epsb[:],
                ).then_inc(dist_sem[b], 1)

        @block.vector
        def _(g):
            # TRN2 sequencers pipeline instruction issue: a DVE instruction
            # can read an operand before the PREVIOUS DVE instruction's write
            # to it has landed.  Tile guards every same-engine RAW/WAR with a
            # per-engine self-semaphore (each op incs it at completion;
            # dependents wait for the producer's count).  Emulate that here.
            state = {"n": 0, "w": {}, "rd": {}}

            def V(ins, reads=(), writes=()):
                ins.then_inc(dve_sem, 1)
                state["n"] += 1
                n = state["n"]
                for t in reads:
                    state["rd"][t] = n
                for t in writes:
                    state["w"][t] = n
                return ins

            def W(reads=(), writes=()):
                # wait for same-engine producers of `reads` and readers/
                # writers of `writes` to complete
                need = 0
                for t in reads:
                    need = max(need, state["w"].get(t, 0))
                for t in writes:
                    need = max(need, state["w"].get(t, 0), state["rd"].get(t, 0))
                if need > 0:
                    g.wait_ge(dve_sem, need)

            V(g.memset(acc6[:], 0.0), writes=[f"a6_{c}" for c in range(N_CLASS)])
            g.memset(epsb[:], EPS).then_inc(eps_sem, 1)  # untracked: ACT-only
            g.wait_ge(tol_sem, 16)
            for k in range(NBLK):
                b, r = k % 2, k // 2
                g.wait_ge(g0_sem[b], 16 * BLK * (r + 1))
                g.wait_ge(g1_sem[b], 16 * BLK * (r + 1))
                g.wait_ge(mk_sem[b], 16 * (r + 1))
                if not do_compute:
                    g.engine_nop().then_inc(ss_sem[b], 1)
                    g.wait_ge(dist_sem[b], r + 1)
                    g.engine_nop().then_inc(cmp_sem[b], 1)
                    continue
                G0 = g0t[:, b].rearrange("p a b c -> p (a b) c")
                G1 = g1t[:, b].rearrange("p a b c -> p (a b) c")
                ssk = f"ss{b}"
                W(writes=["dx"])
                V(g.tensor_sub(out=dxb[:], in0=G0[:, :, 0], in1=G1[:, :, 0]),
                  writes=["dx"])
                W(writes=["dy"])
                V(g.tensor_sub(out=dyb[:], in0=G0[:, :, 1], in1=G1[:, :, 1]),
                  writes=["dy"])
                W(writes=["dz"])
                V(g.tensor_sub(out=dzb[:], in0=G0[:, :, 2], in1=G1[:, :, 2]),
                  writes=["dz"])
                W(writes=["rs"])
                V(g.tensor_add(out=rsb[:], in0=G0[:, :, 3], in1=G1[:, :, 3]),
                  writes=["rs"])
                W(reads=["dx"], writes=[ssk])
                V(g.tensor_mul(out=ssb[:, b], in0=dxb[:], in1=dxb[:]),
                  reads=["dx"], writes=[ssk])
                W(reads=["dy"], writes=["t2y"])
                V(g.tensor_mul(out=t2y[:], in0=dyb[:], in1=dyb[:]),
                  reads=["dy"], writes=["t2y"])
                W(reads=["dz"], writes=["t2z"])
                V(g.tensor_mul(out=t2z[:], in0=dzb[:], in1=dzb[:]),
                  reads=["dz"], writes=["t2z"])
                W(reads=[ssk, "t2y"], writes=[ssk])
                V(g.tensor_add(out=ssb[:, b], in0=ssb[:, b], in1=t2y[:]),
                  reads=["t2y"], writes=[ssk])
                W(reads=[ssk, "t2z"], writes=[ssk])
                # carries ss_sem for ACT (single-update limit); untracked by
                # dve_sem — the next same-parity writer of ssb[b] is two
                # blocks later, transitively ordered via dist_sem.
                g.tensor_add(out=ssb[:, b], in0=ssb[:, b], in1=t2z[:]).then_inc(
                    ss_sem[b], 1
                )
                g.wait_ge(dist_sem[b], r + 1)
                W(reads=["rs"], writes=["base"])
                V(g.tensor_sub(out=baseb[:], in0=rsb[:], in1=distb[:, b]),
                  reads=["rs"], writes=["base"])
                for c in range(N_CLASS):
                    W(reads=["base"], writes=["rc"])
                    V(g.tensor_scalar(
                        out=rcb[:],
                        in0=baseb[:],
                        scalar1=tolb[:, c : c + 1],
                        scalar2=0.0,
                        op0=mybir.AluOpType.add,
                        op1=mybir.AluOpType.max,
                    ), reads=["base"], writes=["rc"])
                    W(writes=["mc"])
                    V(g.tensor_scalar(
                        out=mcb[:],
                        in0=mkt[:, b],
                        scalar1=1 << c,
                        scalar2=None,
                        op0=mybir.AluOpType.bitwise_and,
                    ), writes=["mc"])
                    W(reads=["rc", "mc"], writes=["scr"])
                    V(g.tensor_tensor(
                        out=scrb[:],
                        in0=rcb[:],
                        in1=mcb[:],
                        op=mybir.AluOpType.mult,
                    ), reads=["rc", "mc"], writes=["scr"])
                    W(reads=["scr", f"a6_{c}"], writes=[f"a6_{c}"])
                    V(g.tensor_add(out=acc6[:, c], in0=acc6[:, c], in1=scrb[:]),
                      reads=["scr"], writes=[f"a6_{c}"])
                # probe: waits for every tracked op so far, then signals the
                # block's buffers free (cmp_sem) on its own completion.
                g.wait_ge(dve_sem, state["n"])
                g.memset(prb[:], 0.0).then_inc(cmp_sem[b], 1)
            for c in range(N_CLASS):
                W(reads=[f"a6_{c}"], writes=["acc"])
                V(g.tensor_reduce(
                    out=acc[:, c : c + 1],
                    in_=acc6[:, c],
                    axis=mybir.AxisListType.X,
                    op=mybir.AluOpType.add,
                ), writes=["acc"])
            g.wait_ge(dve_sem, state["n"])
            g.memset(prb[:], 0.0).then_inc(fin_sem, 1)

    nc.compile()
    return nc


_NC_CACHE = {}


def _get_nc():
    if "nc" not in _NC_CACHE:
        _NC_CACHE["nc"] = build_nc()
    return _NC_CACHE["nc"]


def _prep_core(a0, a1, masks6):
    """Sort one core's pairs into (chunk0, chunk1) groups, pad to fixed
    per-group capacity, and emit device layouts."""
    n = a0.shape[0]
    grp = (a0 // CHUNK) * N_CHUNK + (a1 // CHUNK)
    order = np.argsort(grp, kind="stable")
    counts = np.bincount(grp, minlength=N_GROUPS)
    if counts.max() > G_CAP:
        raise RuntimeError(
            f"group capacity exceeded: max count {counts.max()} > {G_CAP}"
        )
    cum = np.zeros(N_GROUPS, dtype=np.int64)
    cum[1:] = np.cumsum(counts)[:-1]
    # position of each sorted pair within its group
    pos = np.arange(n, dtype=np.int64) - np.repeat(cum, counts)
    slot = grp[order] * np.int64(G_CAP) + pos

    idx0 = np.zeros(S_TOT, dtype=np.int16)
    idx1 = np.zeros(S_TOT, dtype=np.int16)
    idx0[slot] = (a0[order] % CHUNK).astype(np.int16)
    idx1[slot] = (a1[order] % CHUNK).astype(np.int16)

    bits = np.zeros(n, dtype=np.uint8)
    for c in range(N_CLASS):
        bits |= masks6[c, order].astype(np.uint8) << c
    mk = np.zeros(S_TOT, dtype=np.uint8)
    mk[slot] = bits

    # queue-banded idx tensor [128, NBLK, 4*CALL_IDXW]: rows [32q, 32q+32)
    # hold queue q's calls (q = 2*endpoint + call_parity), with the wrapped
    # [16, 56] per-call block duplicated for the tx/rx Q7 cores
    idx_dev = np.zeros((128, NBLK, (BLK // 2) * CALL_IDXW), dtype=np.int16)
    for e, x in ((0, idx0), (1, idx1)):
        # [16, k, j, cols] wrapped per call
        w = (
            x.reshape(CALLS_EP, CALL_IDXW, 16)
            .transpose(2, 0, 1)
            .reshape(16, NBLK, BLK, CALL_IDXW)
        )
        for par in range(2):
            band = 32 * (2 * e + par)
            v = w[:, :, par::2, :].reshape(16, NBLK, (BLK // 2) * CALL_IDXW)
            idx_dev[band : band + 16] = v
            idx_dev[band + 16 : band + 32] = v

    # packed mask bytes in per-block layout: [128, NBLK, BLK*CALL_COLS];
    # within a call, slot r -> partition r%128, column-group r//128
    mk_dev = np.ascontiguousarray(
        mk.reshape(NBLK, BLK, CALL_COLS, 128)
        .transpose(3, 0, 1, 2)
        .reshape(128, NBLK, BLK * CALL_COLS)
    )
    return {"idx": idx_dev, "mk": mk_dev}


def _prep_inputs(coords, radii, tollerances, weight, atom_names, atom_pairs, clash_masks):
    """Host-side shard/layout prep. Returns (in_maps, exp_weight)."""
    coords = np.asarray(coords, dtype=np.float32)
    radii = np.asarray(radii, dtype=np.float32)
    tollerances = np.asarray(tollerances, dtype=np.float32)
    atom_names = np.asarray(atom_names).astype(np.int64)
    atom_pairs = np.asarray(atom_pairs).astype(np.int64)
    clash_masks = np.asarray(clash_masks)

    table = np.zeros((N_ATOMS, 4), dtype=np.float32)
    table[:, :3] = coords
    table[:, 3] = radii[atom_names]

    toll2d = np.ascontiguousarray(
        np.broadcast_to(tollerances.reshape(1, N_CLASS), (128, N_CLASS))
    )

    in_maps = []
    for c in range(N_CORES):
        lo, hi = c * PAIRS_PER_CORE, (c + 1) * PAIRS_PER_CORE
        m = _prep_core(
            atom_pairs[lo:hi, 0], atom_pairs[lo:hi, 1], clash_masks[:, lo:hi]
        )
        m["tbl"] = table
        m["toll"] = toll2d
        in_maps.append(m)
    return in_maps, float(np.exp(np.float64(np.asarray(weight).reshape(-1)[0])))


def _finalize(outs, wscale):
    """outs: list of per-core [128, 6] partials. Fold partitions, the 2^c
    mask-bit scale, and exp(weight)."""
    total = np.zeros(N_CLASS, dtype=np.float64)
    for o in outs:
        total += np.asarray(o, dtype=np.float64).reshape(128, N_CLASS).sum(axis=0)
    total /= np.exp2(np.arange(N_CLASS, dtype=np.float64))
    return (total * wscale).astype(np.float32)


def kernel(coords, radii, tollerances, weight, atom_names, atom_pairs, clash_masks):
    nc = _get_nc()
    in_maps, wscale = _prep_inputs(
        coords, radii, tollerances, weight, atom_names, atom_pairs, clash_masks
    )
    res = run_bass_kernel_spmd(nc, in_maps, core_ids=list(range(N_CORES)))
    return _finalize([res.results[c]["out"] for c in range(N_CORES)], wscale)
